# revision 13
# baseline (speedup 1.0000x reference)
"""Multi-head attention (B=1, S=4096, H=16, D=64) on 8 Trainium2 NeuronCores.

Sharding: 2 heads per core (pure head-parallel, no cross-core comms).

v3 vs the 289us baseline -- three structural changes:

1. exp split across TWO engines.  ScalarE keeps query-columns [0, QA) of
   every transposed-score tile (native exp, PSUM->bf16).  DVE handles
   [QA, 512) with a custom fused DVE op (SCHRAUD_EXP_ANT, registered at
   runtime): a phase-corrected Schraudolph producing bf16 *bit patterns*
     t = psum + B;  psi = t - round128(t);  bits = t - min(0.22|psi|, 9.6)
   written as int16, bitcast to bf16 (max wobble +-0.74% measured on hw,
   same class as bf16 rounding).  The constant scale offset is matched
   by ScalarE's exp bias so both engines emit probs = exp(score/8)*G.
   The 128*log2(e)/8 factor is folded into the K prep cast so QK psum
   is already in bit units.

2. All transposes ride the DMA crossbar (dma_start_transpose, 2-byte,
   one batched 3-D instruction per 512-block), not the PE:
   - prep: cast block [128seq, 4, 128d] -> QT/KT[d, seq] directly;
   - drain: oT [65,512] is copied to an 80-row padded fp16 tile
     (ScalarE), crossbar-transposed to [128q, 4, 80], then DVE does one
     reciprocal + one broadcast multiply; the store DMA rides gpsimd's
     queue.  The PE runs nothing but QK/PV matmuls in steady state.

3. Softmax denominators still come free via V's ones-column (row 64 of
   oT); dual-head QK on disjoint PE row groups, PV per (chunk, head)
   accumulating oT in PSUM -- as in the baseline.
"""

import sys

for _p in ("/opt/trn_rl_repo", "/root/.axon_site/_ro/trn_rl_repo"):
    if _p not in sys.path:
        sys.path.append(_p)

import numpy as np

_B, _S, _H, _D = 1, 4096, 16, 64
_NCORES = 8
_HPC = _H // _NCORES  # heads per core

# --- exp split constants ---------------------------------------------------
QA = 224                 # ScalarE query-columns per head per step (of 512)
A_BITS = 128.0 / np.log(2.0) / 8.0      # 23.0831: score -> bf16 bit units
B_BITS = 15758.75        # bit bias (includes +3.75 phase centering)
K_CORR = 0.22            # |psi| slope of the wobble correction
CAP_CORR = 9.6           # correction cap
M_ROUND = 1.5 * (2.0 ** 30)             # round-to-multiple-of-128 magic
G_PROBS = 0.0680999      # measured: DVE probs = exp(score/8) * G_PROBS
ACT_SCALE = 1.0 / (A_BITS * 8.0)        # psum(bit units) -> score/8
ACT_BIAS = float(np.log(G_PROBS))       # match ScalarE probs to DVE scale


def make_exp_op():
    """Register (once) the fused Schraudolph-exp DVE op and return it."""
    from concourse.dve_spec import (
        Spec, Src0, C0, C1, C2, C3, Bin, AluOp, minn, _spill_c3_to_src1, lower,
    )
    from concourse.dve_uop import DveOpSpec
    from concourse import dve_ops as dvo

    name = "SCHRAUD_EXP_ANT"
    for op in dvo.OPS:
        if op.name == name:
            return op
    t = Src0 + C0
    v = t + C3
    i = v - C3
    psi = t - i
    a = Bin(AluOp.ABSOLUTE_VALUE, psi, psi)
    body = _spill_c3_to_src1(t - minn(a * C1, C2))

    def ref(in0, in1, c0, c1, c2):
        f32 = np.float32
        t = (in0.astype(f32) + f32(c0)).astype(f32)
        m = np.asarray(in1, f32).reshape(in1.shape[0], -1)[:, :1]
        m = m.reshape((m.shape[0],) + (1,) * (t.ndim - 1))
        vv = (t + m).astype(f32)
        ii = (vv - m).astype(f32)
        ps = (t - ii).astype(f32)
        corr = np.minimum((np.abs(ps) * f32(c1)).astype(f32), f32(c2))
        return (t - corr).astype(f32)

    spec = Spec(body=body, reference=ref)
    row = max(dvo._SUB_OPCODE_FOR_NAME.values()) + 1
    assert row < 0x20
    uops = lower(spec, ver="v3")
    sha = DveOpSpec(name=name, opcode=row, uops=uops, rd1_en=True).sha("v3")
    op = dvo.DveOp(name, spec, subdim=False, uops_sha={"v3": sha})
    dvo.OPS.append(op)
    dvo._SUB_OPCODE_FOR_NAME[name] = row
    dvo.CUSTOM_DVE_SPECS[name] = spec
    return op


def build_program(S=_S, n_heads=_HPC, blk=512, qa=QA):
    """Build the single-core Bass program (SPMD: same program on all cores)."""
    import concourse.tile as tile
    from concourse import bacc, mybir
    from concourse.masks import make_identity

    exp_op = make_exp_op()

    f32 = mybir.dt.float32
    bf16 = mybir.dt.bfloat16
    f16 = mybir.dt.float16
    i16 = mybir.dt.int16
    D = _D
    W = n_heads * D  # per-core hidden width (128)
    n_sk = S // 128  # key chunks
    n_blk = S // blk  # query superblocks
    n_j = blk // 128
    assert n_heads == 2 and W == 128 and blk % 128 == 0 and n_sk % 4 == 0

    nc = bacc.Bacc("TRN2", target_bir_lowering=False, debug=False)
    q_in = nc.dram_tensor("q", [S, W], f32, kind="ExternalInput")
    k_in = nc.dram_tensor("k", [S, W], f32, kind="ExternalInput")
    v_in = nc.dram_tensor("v", [S, W], f32, kind="ExternalInput")
    out = nc.dram_tensor("out", [S, W], f32, kind="ExternalOutput")

    with tile.TileContext(nc) as tc:
        with (
            tc.tile_pool(name="singles", bufs=1) as singles,
            tc.tile_pool(name="ld", bufs=4) as ld,
            tc.tile_pool(name="qkt", bufs=1) as qkt,
            tc.tile_pool(name="vp", bufs=1) as vpp,
            tc.tile_pool(name="expool", bufs=4) as expool,
            tc.tile_pool(name="osb", bufs=2) as osb,
            tc.tile_pool(name="outb", bufs=2) as outb,
            tc.tile_pool(name="small", bufs=4) as small,
            tc.tile_pool(name="ps_s", bufs=2, space="PSUM") as ps_scores,
            tc.tile_pool(name="ps_o", bufs=1, space="PSUM") as ps_out,
            tc.tile_pool(name="ps_t", bufs=1, space="PSUM") as ps_tp,
        ):
            ident128_bf = singles.tile([128, 128], bf16)
            make_identity(nc, ident128_bf)

            # magic constant for the custom DVE op's round-to-128 trick
            m_round = singles.tile([128, 1], f32, tag="mrnd")
            nc.vector.memset(m_round, float(M_ROUND))
            act_bias = singles.tile([128, 1], f32, tag="actb")
            nc.vector.memset(act_bias, float(ACT_BIAS))

            # Preload the ScalarE exp table set (~1.3us) off the critical
            # path: the first real exp would otherwise pay it.
            dum = small.tile([128, 1], f32, tag="rec1", name="dum")
            nc.vector.memset(dum, 0.0)
            dum2 = small.tile([128, 1], f32, tag="rec1", name="dum2")
            nc.scalar.activation(dum2, dum, mybir.ActivationFunctionType.Exp)

            # PE warmup: dependency-free matmuls at kernel start so the HAM
            # clock-gate opens before real work arrives.
            warm = ps_tp.tile([128, 128], bf16, tag="tp", name="warm")
            for _ in range(10):
                nc.tensor.transpose(warm, ident128_bf, ident128_bf)

            # ---- prep ----
            # QT/KT: [128, S] bf16, head h's d-dims on partitions h*64..+64.
            # KT is pre-scaled by A_BITS so QK psum is in bf16 bit units.
            # V' for both heads in one tensor: [128, n_sk, 130]; head h's
            # 65-wide slab (64 v-dims + ones col) is [:, c, h*65:+65].
            QT = qkt.tile([W, S], bf16, tag="qt")
            KT = qkt.tile([W, S], bf16, tag="kt")
            VP = vpp.tile([128, n_sk, 65 * n_heads], bf16, tag="vp")
            nc.vector.memset(
                VP.rearrange("p c (h x) -> p c h x", x=65)[:, :, :, 64:65], 1.0
            )

            def emit_qk_prep(src, dstT, i4, eng, scale=None, dma_eng=None,
                             fine=False):
                """One 512-row block: DMA in, cast (opt. scaled) to bf16,
                crossbar-transpose into dstT.  fine=True pipelines at
                128-row granularity (for the first blocks on the critical
                path)."""
                units = range(4) if fine else [None]
                tg = "" if not fine else ("fq" if dma_eng is not None else "fk")
                for u0 in units:
                    usl = slice(0, 4) if u0 is None else slice(u0, u0 + 1)
                    nu = 4 if u0 is None else 1
                    rows = slice(i4 * 512 + usl.start * 128,
                                 i4 * 512 + usl.start * 128 + nu * 128)
                    t_ld = ld.tile([128, nu, W], f32, tag="qk_ld" + tg,
                                   name=f"ld_{i4}_{u0}")
                    (dma_eng or nc.sync).dma_start(
                        out=t_ld,
                        in_=src[rows, :].rearrange("(u p) w -> p u w", p=128),
                    )
                    t_bf = ld.tile([128, nu, W], bf16, tag="qk_bf" + tg,
                                   name=f"bf_{i4}_{u0}")
                    if scale is None:
                        eng.tensor_copy(t_bf, t_ld)
                    else:
                        eng.tensor_scalar_mul(t_bf, t_ld, scale)
                    dst = dstT[:, rows].rearrange("p (u s) -> p u s", s=128)
                    nc.sync.dma_start_transpose(out=dst, in_=t_bf)

            # K rides the sync HWDGE queue (first superblock needs all of K
            # early); Q and V load via the gpsimd-triggered queue.  The
            # first K and Q blocks pipeline at 128-row granularity.
            for i4 in range(n_sk // 4):
                rows = slice(i4 * 512, (i4 + 1) * 512)
                if i4 == 0:
                    emit_qk_prep(q_in, QT, 0, nc.gpsimd, dma_eng=nc.gpsimd,
                                 fine=True)
                emit_qk_prep(k_in, KT, i4, nc.vector, scale=float(A_BITS),
                             fine=(i4 == 0))
                v_ld = ld.tile([128, 4, W], f32, tag="v_ld", name=f"vld_{i4}")
                nc.gpsimd.dma_start(
                    out=v_ld,
                    in_=v_in[rows, :].rearrange("(u p) w -> p u w", p=128),
                )
                vdst = VP[:, i4 * 4 : (i4 + 1) * 4, :].rearrange(
                    "p u (h x) -> p u h x", x=65
                )[:, :, :, 0:64]
                vsrc = v_ld.rearrange("p u (h x) -> p u h x", x=64)
                nc.gpsimd.tensor_copy(vdst, vsrc)
            deferred_q = list(range(1, n_sk // 4))

            # ---- main: flat software pipeline over (superblock, chunk).
            # Each step: chunk c's QK for BOTH heads (row offsets 0/64,
            # concurrent) -> one [128, 2, 512] psum tile -> exp split
            # ScalarE/DVE -> two PV accumulations.  QK is emitted 2 steps
            # ahead so neither exp engine waits.
            steps = [(b, c) for b in range(n_blk) for c in range(n_sk)]
            ps_tiles = {}

            def emit_qk(b, c):
                ps = ps_scores.tile(
                    [128, 2, blk], f32, tag="ps", name=f"ps_{b}_{c}"
                )
                ps_tiles[(b, c)] = ps
                for h in range(n_heads):
                    p0 = h * 64
                    nc.tensor.matmul(
                        ps[:, h, :],
                        lhsT=KT[p0 : p0 + 64, c * 128 : (c + 1) * 128],
                        rhs=QT[p0 : p0 + 64, b * blk : (b + 1) * blk],
                        start=True,
                        stop=True,
                    )

            def emit_exp(ps, ex):
                # ScalarE: native exp on cols [0, qa) of both heads.
                nc.scalar.activation(
                    ex[:, :, 0:qa], ps[:, :, 0:qa],
                    mybir.ActivationFunctionType.Exp,
                    scale=float(ACT_SCALE), bias=act_bias,
                )
                # DVE: Schraudolph bit-trick on cols [qa, 512).
                nc.vector._custom_dve(
                    exp_op,
                    out=ex[:, :, qa:blk].bitcast(i16),
                    in0=ps[:, :, qa:blk],
                    in1=m_round,
                    s0=float(B_BITS), s1=float(K_CORR), imm2=float(CAP_CORR),
                )

            # Drain a finished (block, head): copy oT to an 80-row fp16
            # tile (ScalarE), crossbar-transpose to q-major, one DVE
            # reciprocal + one broadcast multiply, store via gpsimd queue.
            def emit_drain(b, h, oT_tile):
                o_sb = osb.tile([80, blk], f16, tag="osb", name=f"osb_{h}_{b}")
                nc.gpsimd.memset(o_sb[64:80, :], 0.0)
                nc.scalar.copy(o_sb[0:65, :], oT_tile)
                obm16 = outb.tile([128, n_j, 80], f16, tag="ob16",
                                  name=f"ob16_{h}_{b}")
                nc.sync.dma_start_transpose(out=obm16, in_=o_sb)
                rec = small.tile([128, n_j, 1], f32, tag="rec",
                                 name=f"rec_{b}_{h}")
                nc.vector.reciprocal(rec[:, :, 0], obm16[:, :, 64])
                obm = outb.tile([128, n_j, 64], f32, tag="obm",
                                name=f"obm_{h}_{b}")
                nc.vector.tensor_tensor(
                    out=obm, in0=obm16[:, :, 0:64],
                    in1=rec.broadcast_to([128, n_j, 64]),
                    op=mybir.AluOpType.mult,
                )
                P0 = h * 64
                nc.gpsimd.dma_start(
                    out=out[b * blk : (b + 1) * blk, P0 : P0 + 64].rearrange(
                        "(j p) d -> p j d", p=128
                    ),
                    in_=obm,
                )

            # Head1's PV stream runs 2 steps behind head0's (frees the
            # single-buffered oT1 slot before reuse at block boundaries);
            # flushed immediately in the last superblock to shorten the
            # tail.
            def emit_pv(h, oT_tile, c, ex_tile):
                nc.tensor.matmul(
                    oT_tile,
                    lhsT=VP[:, c, h * 65 : (h + 1) * 65],
                    rhs=ex_tile[:, h, :],
                    start=(c == 0),
                    stop=(c == n_sk - 1),
                )

            emit_qk(*steps[0])
            emit_qk(*steps[1])
            oT0 = None
            oT1_by_b = {}
            pend_h1 = []
            for idx, (b, c) in enumerate(steps):
                last_blk = b == n_blk - 1
                if c == 0:
                    oT0 = ps_out.tile(
                        [65, blk], f32, tag="oT0", name=f"oT_0_{b}", bufs=2
                    )
                ps = ps_tiles.pop((b, c))
                ex = expool.tile([128, 2, blk], bf16, tag="ex", name=f"ex_{idx}")
                emit_exp(ps, ex)
                if idx + 2 < len(steps):
                    emit_qk(*steps[idx + 2])
                if deferred_q and deferred_q[0] == b + 1 and c == min(20, n_sk - 4):
                    emit_qk_prep(q_in, QT, deferred_q.pop(0), nc.gpsimd,
                                 dma_eng=nc.gpsimd)
                emit_pv(0, oT0, c, ex)
                pend_h1.append((b, c, ex))
                while pend_h1 and (len(pend_h1) > 2 or (last_blk and c >= 2)):
                    b1, c1, ex1 = pend_h1.pop(0)
                    if c1 == 0:
                        oT1_by_b[b1] = ps_out.tile(
                            [65, blk], f32, tag="oT1", name=f"oT_1_{b1}", bufs=1
                        )
                    emit_pv(1, oT1_by_b[b1], c1, ex1)
                    if c1 == n_sk - 1:
                        emit_drain(b1, 1, oT1_by_b.pop(b1))
                if c == n_sk - 1:
                    emit_drain(b, 0, oT0)
            for b1, c1, ex1 in pend_h1:
                if c1 == 0:
                    oT1_by_b[b1] = ps_out.tile(
                        [65, blk], f32, tag="oT1", name=f"oT_1_{b1}", bufs=1
                    )
                emit_pv(1, oT1_by_b[b1], c1, ex1)
                if c1 == n_sk - 1:
                    emit_drain(b1, 1, oT1_by_b.pop(b1))
            assert not deferred_q
    nc.finalize()
    return nc


def _shard_inputs(query, key, value):
    """Full [1, S, H*D] inputs -> per-core [S, HPC*D] contiguous column blocks."""
    w = _HPC * _D
    in_maps = []
    for c in range(_NCORES):
        sl = slice(c * w, (c + 1) * w)
        in_maps.append(
            {
                "q": np.ascontiguousarray(query[0, :, sl]),
                "k": np.ascontiguousarray(key[0, :, sl]),
                "v": np.ascontiguousarray(value[0, :, sl]),
            }
        )
    return in_maps


def kernel(query, key, value, trace=False, tmpdir=None):
    from concourse.bass_utils import run_bass_kernel_spmd

    query = np.asarray(query, dtype=np.float32)
    key = np.asarray(key, dtype=np.float32)
    value = np.asarray(value, dtype=np.float32)

    nc = build_program()
    in_maps = _shard_inputs(query, key, value)
    res = run_bass_kernel_spmd(
        nc, in_maps, list(range(_NCORES)), trace=trace, tmpdir=tmpdir
    )
    full = np.concatenate([res.results[c]["out"] for c in range(_NCORES)], axis=1)
    out = full[None].astype(np.float32)
    if trace:
        return out, res
    return out
